# revision 39
# baseline (speedup 1.0000x reference)
"""AnchorAttention distributed Bass kernel for 8 TRN2 NeuronCores.

Reference computation (B=2, S=4096, D=1024, H=16, Dh=64, A=512):
  anchors = x[:, :A];  queries = x[:, A:]
  anchor_q/k/v = split_heads(anchors @ Wq/Wk/Wv + b)
  query_q      = split_heads(queries @ Wqt + bqt)
  combined_q   = concat([anchor_q, query_q], axis=2)       # [B,H,S,Dh]
  out  = softmax(combined_q @ anchor_k^T / sqrt(Dh)) @ anchor_v
  out  = merge_heads(out) @ Wo + bo

Sharding: the B*S = 8192 token rows are split into 8 chunks of 1024 rows
(core c -> batch c//4, rows (c%4)*1024 ...). Each core duplicates its
batch's anchor K/V projections, computes Q for its own rows (Wq for the
anchor-region rows, Wqt for query rows), attention over the 512 anchors
for all 16 heads, and the output projection for its rows. The output is a
pure concatenation: no collectives.

Layout: everything is kept transposed ([feature, row]) so each matmul
contracts over the partition dim with zero on-chip transposes; the final
output projection naturally lands un-transposed [row, feature] for DMA
out. Host pre-transposes/pre-casts inputs to bf16 (compute dtype; f32
accumulation in PSUM). Softmax row-sums come free via an extra all-ones
column appended to V; no max-subtraction is needed (scores are ~N(0,1),
exp stays in a tiny range; softmax is shift-invariant so results match).

Heads are packed two per 128-partition tile (head h -> column-tile h//2,
partitions (h%2)*64 ..). The odd head of each pair has its V-slab ones
column *first* so the AV output [sums; attn^T] fits partitions 63..127.
"""

from contextlib import ExitStack

import numpy as np
import ml_dtypes

import concourse.bass as bass
import concourse.tile as tile
from concourse import bacc, mybir
from concourse import bass_utils

BF16 = mybir.dt.bfloat16
F32 = mybir.dt.float32
F32R = mybir.dt.float32r

B, S, D = 2, 4096, 1024
H, DH = 16, 64
A = 512                  # num_anchor_tokens (asserted at runtime)
RPC = 1024               # rows per core
NCORES = 8
SCALE = 1.0 / np.sqrt(float(DH))

_CACHE = {}


def _build():
    """Build + compile the per-core Bass graph (identical on all cores)."""
    nc = bacc.Bacc("TRN2", target_bir_lowering=False, debug=False)

    xt = nc.dram_tensor("xt", [128, 8, RPC], BF16, kind="ExternalInput")   # rows^T swizzled
    at = nc.dram_tensor("at", [128, 8, A], BF16, kind="ExternalInput")     # anchors^T swizzled
    wlo = nc.dram_tensor("wlo", [128, 8, D], BF16, kind="ExternalInput")   # Q weight rows 0-511
    whi = nc.dram_tensor("whi", [128, 8, D], BF16, kind="ExternalInput")   # Q weight rows 512-1023
    wk = nc.dram_tensor("wk", [128, 8, D], BF16, kind="ExternalInput")
    wv = nc.dram_tensor("wv", [128, 8, D], BF16, kind="ExternalInput")
    wo = nc.dram_tensor("wo", [128, 8, D], BF16, kind="ExternalInput")
    blo = nc.dram_tensor("blo", [128, 8], F32, kind="ExternalInput")
    bhi = nc.dram_tensor("bhi", [128, 8], F32, kind="ExternalInput")
    bk = nc.dram_tensor("bk", [128, 8], F32, kind="ExternalInput")
    bv = nc.dram_tensor("bv", [128, D], F32, kind="ExternalInput")   # pre-broadcast
    bo = nc.dram_tensor("bo", [128, D], F32, kind="ExternalInput")   # pre-broadcast
    out = nc.dram_tensor("out", [RPC, D], F32, kind="ExternalOutput")

    Exp = mybir.ActivationFunctionType.Exp

    with tile.TileContext(nc) as tc:
        with tc.tile_pool(name="wpool", bufs=1) as wpool, \
             tc.tile_pool(name="cpool", bufs=1) as cpool, \
             tc.tile_pool(name="kvpool", bufs=1) as kvpool, \
             tc.tile_pool(name="qtpool", bufs=2) as qtpool, \
             tc.tile_pool(name="psum", bufs=2, space="PSUM") as psum:
            # Q weights + x slabs live only through the projection phases;
            # their pools close before the attention pools open so the
            # attention working set reuses their SBUF space.
            projstack = ExitStack()
            wqpool = projstack.enter_context(tc.tile_pool(name="wqpool", bufs=1))
            xpool = projstack.enter_context(tc.tile_pool(name="xpool", bufs=1))

            # ---- input DMAs; host pre-swizzles to [128, 8, cols] so each
            # partition's slab row is one contiguous 16KB DMA descriptor ----
            def slab_in(pool, t, cols, name):
                s = pool.tile([128, 8, cols], BF16, name=name)
                nc.sync.dma_start(out=s, in_=t.ap())
                return s

            wk_sb = slab_in(wpool, wk, D, "wk_sb")
            at_sb = slab_in(xpool, at, A, "at_sb")
            xt_sb = slab_in(xpool, xt, RPC, "xt_sb")
            wlo_sb = slab_in(wqpool, wlo, D, "wlo_sb")
            wv_sb = slab_in(wpool, wv, D, "wv_sb")
            whi_sb = slab_in(wqpool, whi, D, "whi_sb")
            wo_sb = slab_in(wpool, wo, D, "wo_sb")

            def bias_in(t, name):  # host pre-arranged [128, 8]
                s = cpool.tile([128, 8], F32, name=name)
                nc.sync.dma_start(out=s, in_=t.ap())
                return s

            blo_sb = bias_in(blo, "blo_sb")
            bhi_sb = bias_in(bhi, "bhi_sb")
            bk_sb = bias_in(bk, "bk_sb")

            def bias_bc(t, name):  # host pre-broadcast [128, D]
                s = cpool.tile([128, D], F32, name=name)
                nc.sync.dma_start(out=s, in_=t.ap())
                return s

            bv_bc = bias_bc(bv, "bv_bc")
            bo_bc = bias_bc(bo, "bo_bc")

            ones_bf = cpool.tile([128, DH], BF16, name="ones_bf")
            nc.vector.memset(ones_bf, 1.0)



            # V slab: [128(a%128), ach, head, 65]; cols 0-63 = V head slice,
            # col 64 = ones (supplies softmax row-sums during AV).
            vaug = kvpool.tile([128, 4, H, DH + 1], BF16, name="vaug")
            nc.vector.memset(vaug, 1.0)

            # ---- K^T projection: kt[c, a] = (anchors @ Wk)^T ----
            kt_sb = kvpool.tile([128, 8, A], BF16, name="kt_sb")
            for ct in range(8):
                pk = psum.tile([128, A], F32, tag="work", name="pk")
                for dt in range(8):
                    nc.tensor.matmul(
                        pk, wk_sb[:, dt, ct * 128:(ct + 1) * 128],
                        at_sb[:, dt, :], start=(dt == 0), stop=(dt == 7))
                nc.vector.tensor_scalar_add(
                    kt_sb[:, ct, :], pk, bk_sb[:, ct:ct + 1])

            # ---- Q^T projection per 512-row chunk, written into two
            # zero-padded slabs (z0: odd-head partitions zeroed, z1: even)
            # so score matmuls contract over the full 128 partitions and the
            # PE never leaves 128-row mode (FWL stays on, no mode-switch
            # drains). V projection slotted between the two chunks to match
            # DMA arrival order. ----
            qts = []
            for rc in range(2):
                wsel = wlo_sb if rc == 0 else whi_sb
                bsel = blo_sb if rc == 0 else bhi_sb
                qt_z0 = qtpool.tile([128, 8, 512], BF16, tag=f"qt0_{rc}",
                                    name=f"qt_z0_{rc}", bufs=1)
                qt_z1 = qtpool.tile([128, 8, 512], BF16, tag=f"qt1_{rc}",
                                    name=f"qt_z1_{rc}", bufs=1)
                nc.vector.memset(qt_z0[64:128, :, :], 0.0)
                nc.vector.memset(qt_z1[0:64, :, :], 0.0)
                for ct in range(8):
                    pq = psum.tile([128, 512], F32, tag="work", name="pq")
                    for dt in range(8):
                        nc.tensor.matmul(
                            pq, wsel[:, dt, ct * 128:(ct + 1) * 128],
                            xt_sb[:, dt, rc * 512:(rc + 1) * 512],
                            start=(dt == 0), stop=(dt == 7))
                    nc.vector.tensor_scalar_add(
                        qt_z0[0:64, ct, :], pq[0:64, :], bsel[0:64, ct:ct + 1])
                    nc.vector.tensor_scalar_add(
                        qt_z1[64:128, ct, :], pq[64:128, :],
                        bsel[64:128, ct:ct + 1])
                qts.append((qt_z0, qt_z1))
                if rc == 0:
                    # V projection (un-transposed): v[a, c] = anchors @ Wv
                    for ach in range(4):
                        for ch in range(2):
                            pv = psum.tile([128, 512], F32, tag="work",
                                           name="pv")
                            for dt in range(8):
                                nc.tensor.matmul(
                                    pv, at_sb[:, dt, ach * 128:(ach + 1) * 128],
                                    wv_sb[:, dt, ch * 512:(ch + 1) * 512],
                                    start=(dt == 0), stop=(dt == 7))
                            pv_v = pv.rearrange("p (hd d) -> p hd d", d=DH)
                            bv_v = bv_bc.rearrange(
                                "p (chd hd d) -> p chd hd d",
                                chd=2, d=DH)[:, ch]
                            nc.vector.tensor_add(
                                vaug[:, ach, ch * 8:(ch + 1) * 8, 0:DH],
                                pv_v, bv_v)

            # ---- attention, software-pipelined over the 8 head-pair
            # groups (ct): scores+exp run one group ahead of AV, two ahead
            # of the normalization; both heads of a group share one praw2
            # slab, one reciprocal, and one [128, 1024] normalize multiply.
            # The 1/sums broadcast is a PE ones-outer-product written into
            # partitions 0-127 of the group's SECOND pav tile (its rows were
            # already evacuated), so no PSUM banks are added and the DVE
            # multiply reads it with mixed partition bases. ----
            projstack.close()
            attnstack = ExitStack()
            attnpool = attnstack.enter_context(tc.tile_pool(name="attnpool", bufs=1))
            ptpool = attnstack.enter_context(tc.tile_pool(name="ptpool", bufs=8))
            tmppool = attnstack.enter_context(tc.tile_pool(name="tmppool", bufs=3))
            rcppool = attnstack.enter_context(tc.tile_pool(name="rcppool", bufs=2))
            outpool = attnstack.enter_context(tc.tile_pool(name="outpool", bufs=3))
            attnT = attnpool.tile([128, 8, RPC], BF16, name="attnT")

            def stage_scores(ct):
                st = {"pts": []}
                for par in range(2):
                    for rc in range(2):
                        qt_sb = qts[rc][par]
                        pt = ptpool.tile([128, 4, 512], BF16, tag="pt",
                                         name="pt")
                        for half in range(2):
                            s2 = psum.tile([128, 2, 512], F32, tag="s",
                                           name="s2", bufs=2)
                            for k in range(2):
                                ach = 2 * half + k
                                nc.tensor.matmul(
                                    s2[:, k, :],
                                    kt_sb[:, ct, ach * 128:(ach + 1) * 128],
                                    qt_sb[:, ct, :],
                                    start=True, stop=True)
                            nc.scalar.activation(
                                out=pt[:, 2 * half:2 * half + 2, :], in_=s2,
                                func=Exp, scale=SCALE)
                        st["pts"].append(pt)
                return st

            def stage_av(ct, par, st):
                h = 2 * ct + par
                pav = psum.tile([128, 2, 512], F32, tag="work", name="pav",
                                bufs=2)
                for rc in range(2):
                    pt = st["pts"][par * 2 + rc]
                    for ach in range(4):
                        nc.tensor.matmul(
                            pav[0:DH + 1, rc, :], vaug[:, ach, h, :],
                            pt[:, ach, :], start=(ach == 0), stop=(ach == 3))
                if par == 0:
                    st["praw2"] = tmppool.tile([128, 2, 512], BF16,
                                               tag="praw", name="praw2")
                    # sums gathered to partition bases {0,64} of one tile
                    # so the reciprocal+cast run lane-parallel
                    st["sums4"] = rcppool.tile([128, 2, 512], F32,
                                               tag="sums", name="sums4")
                nc.any.tensor_copy(st["praw2"][par * 64:par * 64 + DH, :, :],
                                   pav[0:DH, :, :])
                row = par * 64
                nc.vector.tensor_copy(st["sums4"][row:row + 1, :, :],
                                      pav[DH:DH + 1, :, :])
                st[f"pav{par}"] = pav

            def stage_recip(ct, st):
                rcp4 = rcppool.tile([128, 2, 512], F32, tag="rcp",
                                    name="rcp4")
                nc.vector.reciprocal_approx_fast(rcp4, st["sums4"])
                rcpbf = rcppool.tile([128, 2, 512], BF16, tag="rcpbf",
                                     name="rcpbf")
                nc.vector.tensor_copy(rcpbf, rcp4)
                st["rcpbf"] = rcpbf

            def stage_norm(ct, st):
                pav1 = st["pav1"]
                for par in range(2):
                    row = par * 64
                    for rcn in range(2):
                        nc.tensor.matmul(
                            pav1[par * 64:(par + 1) * 64, rcn, :],
                            ones_bf[row:row + 1, :],
                            st["rcpbf"][row:row + 1, rcn, :],
                            start=True, stop=True)
                dst = attnT[:, ct, :].rearrange("p (b r) -> p b r", b=2)
                nc.vector.tensor_mul(dst, st["praw2"], pav1)

            sts = {}
            for i in range(10):
                if i < 8:
                    sts[i] = stage_scores(i)
                if 1 <= i <= 8:
                    stage_av(i - 1, 0, sts[i - 1])
                if 2 <= i <= 9:
                    stage_recip(i - 2, sts[i - 2])
                    stage_norm(i - 2, sts[i - 2])
                if 1 <= i <= 8:
                    stage_av(i - 1, 1, sts[i - 1])

            # ---- output projection ----
            for rti in range(8):
                for nh in range(2):
                    pout = psum.tile([128, 512], F32, tag="work",
                                     name="pout")
                    for ct2 in range(8):
                        nc.tensor.matmul(
                            pout, attnT[:, ct2, rti * 128:(rti + 1) * 128],
                            wo_sb[:, ct2, nh * 512:(nh + 1) * 512],
                            start=(ct2 == 0), stop=(ct2 == 7))
                    out_t = outpool.tile([128, 512], F32, tag="out",
                                         name="out_t")
                    nc.vector.tensor_add(out_t, pout,
                                         bo_bc[:, nh * 512:(nh + 1) * 512])
                    nc.sync.dma_start(
                        out=out.ap()[rti * 128:(rti + 1) * 128,
                                     nh * 512:(nh + 1) * 512],
                        in_=out_t)
            attnstack.close()

    nc.compile()
    return nc


def _swz(a):
    """[1024, cols] -> [128, 8, cols] with row r -> (r % 128, r // 128)."""
    return np.ascontiguousarray(
        a.reshape(8, 128, -1).transpose(1, 0, 2))


def _make_in_maps(x, Wq, bq, Wk, bk, Wv, bv, Wqt, bqt, Wo, bo):
    x = np.asarray(x, dtype=np.float32)
    bf = ml_dtypes.bfloat16

    wq_b = np.ascontiguousarray(np.asarray(Wq, np.float32).astype(bf))
    wqt_b = np.ascontiguousarray(np.asarray(Wqt, np.float32).astype(bf))
    wk_b = np.ascontiguousarray(np.asarray(Wk, np.float32).astype(bf))
    wv_b = np.ascontiguousarray(np.asarray(Wv, np.float32).astype(bf))
    wo_b = np.ascontiguousarray(np.asarray(Wo, np.float32).astype(bf))
    colmaj = lambda v: np.ascontiguousarray(
        np.asarray(v, np.float32).reshape(8, 128).T)
    bq, bqt, bk = map(colmaj, (bq, bqt, bk))
    bv = np.ascontiguousarray(
        np.broadcast_to(np.asarray(bv, np.float32), (128, D)))
    bo = np.ascontiguousarray(
        np.broadcast_to(np.asarray(bo, np.float32), (128, D)))

    wq_sw, wqt_sw = _swz(wq_b), _swz(wqt_b)
    wk_sw, wv_sw, wo_sw = _swz(wk_b), _swz(wv_b), _swz(wo_b)
    at_sw = [_swz(x[b, :A, :].T.astype(bf)) for b in range(B)]
    in_maps = []
    for c in range(NCORES):
        b, q = divmod(c, 4)
        rows = x[b, q * RPC:(q + 1) * RPC, :]
        in_maps.append({
            "xt": _swz(rows.T.astype(bf)),
            "at": at_sw[b],
            "wlo": wq_sw if q == 0 else wqt_sw,
            "whi": wqt_sw,
            "wk": wk_sw, "wv": wv_sw, "wo": wo_sw,
            "blo": bq if q == 0 else bqt, "bhi": bqt,
            "bk": bk, "bv": bv, "bo": bo,
        })
    return in_maps


def kernel(x, Wq, bq, Wk, bk, Wv, bv, Wqt, bqt, Wo, bo, num_anchor_tokens):
    assert int(num_anchor_tokens) == A
    if "nc" not in _CACHE:
        _CACHE["nc"] = _build()
    nc = _CACHE["nc"]

    in_maps = _make_in_maps(x, Wq, bq, Wk, bk, Wv, bv, Wqt, bqt, Wo, bo)
    res = bass_utils.run_bass_kernel_spmd(
        nc, in_maps, core_ids=list(range(NCORES)))
    out = np.empty((B, S, D), np.float32)
    for c in range(NCORES):
        b, q = divmod(c, 4)
        out[b, q * RPC:(q + 1) * RPC, :] = res.results[c]["out"]
    return out
